# revision 17
# baseline (speedup 1.0000x reference)
"""Trainium2 Bass/Tile kernel for ExtAttentionPool (nn_ExtAttentionPool).

Math (per sample b):
    S[u, o]  = sum_d L[u, d] * W[o, d]
    E[o, u]  = exp(S[u,o]/O + b[o]/O)          (softmax numerator over u)
    Z[o]     = sum_u E[o, u]
    OUT[o,t] = (1/Z[o]) * sum_c E[o, c] * L[t, c]
    result row b = OUT flattened (O-major), shape (O*T,)

Sharding: data-parallel over batch B=16 across 8 cores (2 samples/core).

Key implementation points:
  - Both matmuls contract over logits' D axis, so logits is transposed
    on-chip as a REGULAR bf16 matmul against an identity moving operand
    (engages fast-weight-load, counts as PE-busy for the HAM clock gate).
  - Logits are cast f32->bf16 inline in the SWDGE DMA load; accumulation
    stays fp32 in PSUM. All chunk DMAs are issued up front; chunks are
    kept <= 1 MiB so arrival gaps stay under the ~3.4 us HAM re-throttle
    window and the PE clock stays at 2.4 GHz.
  - mm1 is emitted in 256-wide column quarters as soon as the t-rows
    feeding a quarter are transposed: it fills PE idle gaps during the
    DMA stream and leaves only one small quarter on the critical tail.
  - mm2's two 512-wide halves run concurrently in PE column groups 0/1
    (tile_position). The 1/Z scaling rides the final PSUM->SBUF copies:
    half 0 scales by rz on ScalarE, half 1 by a partition-shifted rz
    (tiny const shift-matrix matmul) on VectorE.
"""

import numpy as np
from contextlib import ExitStack

import concourse.bass as bass
import concourse.mybir as mybir
import concourse.tile as tile
from concourse import bacc
from concourse.bass_utils import run_bass_kernel_spmd
from concourse.masks import make_identity

F32 = mybir.dt.float32
BF16 = mybir.dt.bfloat16

N_CORES = 8
B_FULL = 16


def build_nc(b_per=2, T=1024, D=1024, O=10, warmup_mms=12):
    """Build the per-core Bass program (bf16 compute). Same on all 8 cores."""
    P = 128
    NT = T // P            # 128-row t-blocks
    ND = D // P            # 128-col d-blocks
    QW = min(T, 256)       # mm1 quarter width
    NQ = T // QW           # mm1 quarters
    NH = max(1, T // 512)  # mm2 512-wide halves
    HW = min(T, 512)
    # per-sample DMA chunk plans (in 128-row blocks); all chunks <= 1 MiB.
    if NT == 8:
        plans = [[1, 1, 2, 2, 2], [2, 2, 2, 1, 1]]
    else:
        plans = [[1] * NT for _ in range(b_per)]

    nc = bacc.Bacc(
        "TRN2", target_bir_lowering=False, debug=False, enable_asserts=False
    )
    logits = nc.dram_tensor("logits", (b_per, T, D), F32, kind="ExternalInput").ap()
    w_in = nc.dram_tensor("W", (O, D), F32, kind="ExternalInput").ap()
    b_in = nc.dram_tensor("b", (O,), F32, kind="ExternalInput").ap()
    out = nc.dram_tensor("out", (b_per, O * T), F32, kind="ExternalOutput").ap()

    n_chunks = sum(len(p) for p in plans)

    with tile.TileContext(nc) as tc, ExitStack() as ctx:
        singles = ctx.enter_context(tc.tile_pool(name="singles", bufs=1))
        lr_pool = ctx.enter_context(tc.tile_pool(name="lr", bufs=n_chunks))
        lt_pool = ctx.enter_context(tc.tile_pool(name="lt", bufs=2))
        e_pool = ctx.enter_context(tc.tile_pool(name="e", bufs=2))
        z_pool = ctx.enter_context(tc.tile_pool(name="z", bufs=2))
        osb_pool = ctx.enter_context(tc.tile_pool(name="osb", bufs=2))
        slab_ps = ctx.enter_context(tc.tile_pool(name="slab", bufs=2, space="PSUM"))
        s_ps = ctx.enter_context(tc.tile_pool(name="sps", bufs=2, space="PSUM"))
        o_ps = ctx.enter_context(tc.tile_pool(name="ops", bufs=1, space="PSUM"))
        et_ps = ctx.enter_context(tc.tile_pool(name="etps", bufs=1, space="PSUM"))
        rep_ps = ctx.enter_context(tc.tile_pool(name="repps", bufs=1, space="PSUM"))

        # --- identity (f32 master + bf16 cast), then chunk DMAs up front ---
        identf = singles.tile([P, P], F32)
        make_identity(nc, identf)
        ident = singles.tile([P, P], BF16)
        nc.vector.tensor_copy(ident, identf)

        max_rj = max(max(p) for p in plans)
        lr_tiles = {}  # (s, chunk_idx) -> (lr_tile, r0, rj, is_f32)
        w_sb = singles.tile([O, D], BF16)
        for s in range(b_per):
            r = 0
            for ci, rj in enumerate(plans[s]):
                f32_chunk = s == 0 and ci < 2
                if f32_chunk:
                    # HWDGE (sync) f32 load: lower first-byte latency, and
                    # the f32 transposes double as HAM warmup.
                    lr = lr_pool.tile(
                        [P, rj, D], F32, tag="lrf", name=f"lrf_s{s}c{ci}"
                    )
                    nc.sync.dma_start(
                        out=lr,
                        in_=logits[
                            s, r * P : (r + rj) * P, :
                        ].rearrange("(j p) d -> p j d", p=P),
                    )
                else:
                    lr = lr_pool.tile(
                        [P, max_rj, D], BF16, tag="lr", name=f"lr_s{s}c{ci}"
                    )
                    nc.gpsimd.dma_start(
                        out=lr[:, :rj, :],
                        in_=logits[
                            s, r * P : (r + rj) * P, :
                        ].rearrange("(j p) d -> p j d", p=P),
                    )
                lr_tiles[(s, ci)] = (lr, r, rj, f32_chunk)
                r += rj
                if s == 0 and ci == 0:
                    # W load early (needed by the WT transposes)
                    nc.gpsimd.dma_start(out=w_sb, in_=w_in)

        b_sb = singles.tile([O, 1], F32)
        nc.sync.dma_start(out=b_sb, in_=b_in.rearrange("(o u) -> o u", u=1))
        bias01 = singles.tile([O, 1], F32)
        nc.scalar.activation(
            out=bias01, in_=b_sb,
            func=mybir.ActivationFunctionType.Copy, scale=1.0 / O,
        )

        # shiftmat[o, m] = 1 iff m == o or m == o + 32 (for rz replication)
        shiftmat = singles.tile([O, 42], F32)
        nc.gpsimd.memset(shiftmat, 0.0)
        nc.gpsimd.affine_select(
            out=shiftmat, in_=shiftmat,
            compare_op=mybir.AluOpType.not_equal, fill=1.0,
            base=0, pattern=[[-1, 42]], channel_multiplier=1,
        )
        nc.gpsimd.affine_select(
            out=shiftmat, in_=shiftmat,
            compare_op=mybir.AluOpType.not_equal, fill=1.0,
            base=32, pattern=[[-1, 42]], channel_multiplier=1,
        )

        # --- PE warmup: identity matmuls to lift the HAM clock gate ---
        warm = slab_ps.tile([P, 4 * P], F32, tag="slab")
        for i in range(warmup_mms):
            k = i % 4
            nc.tensor.matmul(
                warm[:, k * P : (k + 1) * P], lhsT=ident, rhs=ident,
                start=True, stop=True,
            )

        # WT[dp, c, o] = W[o, 128c+dp]  (regular-matmul transpose)
        wt_stage = et_ps.tile([P, ND, O], F32, tag="etps")
        for c in range(ND):
            nc.tensor.matmul(
                wt_stage[:, c, :],
                lhsT=w_sb[:, c * P : (c + 1) * P],
                rhs=ident[:O, :O],
                start=True, stop=True,
            )
        wt_sb = singles.tile([P, ND, O], BF16)
        nc.vector.tensor_copy(wt_sb, wt_stage)

        i_copy = [0]

        def transpose_blocks(lr, lt, j, r, is_f32=False):
            """PE-transpose row-block r (from lr slot j) into lt."""
            rhs_id = identf if is_f32 else ident
            for g in range(ND // 4):
                slab = slab_ps.tile([P, 4 * P], F32, tag="slab")
                for k in range(4):
                    c = 4 * g + k
                    nc.tensor.matmul(
                        slab[:, k * P : (k + 1) * P],
                        lhsT=lr[:, j, c * P : (c + 1) * P],
                        rhs=rhs_id,
                        start=True, stop=True,
                    )
                dst = lt[:, 4 * g : 4 * g + 4, r * P : (r + 1) * P]
                if i_copy[0] % 2 == 0:
                    nc.vector.tensor_copy(dst, slab)
                else:
                    nc.scalar.activation(
                        out=dst, in_=slab,
                        func=mybir.ActivationFunctionType.Copy,
                    )
                i_copy[0] += 1

        def process_sample(s):
            lt = lt_pool.tile([P, ND, T], BF16, tag="lt")
            e_sb = e_pool.tile([O, T], BF16, tag="e")
            zparts = z_pool.tile([O, NQ], F32, tag="z")
            et_stage = et_ps.tile([P, ND, O], F32, tag="etps")
            ec = e_pool.tile([P, ND, O], BF16, tag="ec")

            r = 0
            q_done = 0
            for ci, rj in enumerate(plans[s]):
                lr, r0, _, is_f32 = lr_tiles[(s, ci)]
                for j in range(rj):
                    transpose_blocks(lr, lt, j, r0 + j, is_f32)
                r += rj
                # mm1 + exp + E-transpose for every 256-wide quarter whose
                # t-rows are now fully transposed
                while q_done < NQ and r * P >= (q_done + 1) * QW:
                    q = q_done
                    sp = s_ps.tile([O, QW], F32, tag="sps", name=f"sp{s}_{q}")
                    for c in range(ND):
                        nc.tensor.matmul(
                            sp,
                            lhsT=wt_sb[:, c, :],
                            rhs=lt[:, c, q * QW : (q + 1) * QW],
                            start=(c == 0),
                            stop=(c == ND - 1),
                        )
                    nc.scalar.activation(
                        out=e_sb[:, q * QW : (q + 1) * QW],
                        in_=sp,
                        func=mybir.ActivationFunctionType.Exp,
                        scale=1.0 / O,
                        bias=bias01,
                        accum_out=zparts[:, q : q + 1],
                    )
                    cpq = QW // P  # E-transpose blocks in this quarter
                    for c in range(q * cpq, (q + 1) * cpq):
                        nc.tensor.matmul(
                            et_stage[:, c, :],
                            lhsT=e_sb[:, c * P : (c + 1) * P],
                            rhs=ident[:O, :O],
                            start=True, stop=True,
                        )
                    nc.vector.tensor_copy(
                        ec[:, q * cpq : (q + 1) * cpq, :],
                        et_stage[:, q * cpq : (q + 1) * cpq, :],
                    )
                    q_done += 1

            # softmax denominator chain
            zsum = z_pool.tile([O, 1], F32, tag="zs")
            nc.vector.reduce_sum(zsum, zparts, axis=mybir.AxisListType.X)
            rz = z_pool.tile([O, 1], F32, tag="rz")
            nc.vector.reciprocal(rz, zsum)
            if NH == 2:
                # replicate rz to partitions 32:32+O for the packed half-1
                rep = rep_ps.tile([42, 1], F32, tag="repps")
                nc.tensor.matmul(rep, lhsT=shiftmat, rhs=rz, start=True, stop=True)
                rz_rep = z_pool.tile([42, 1], F32, tag="rzrep")
                nc.vector.tensor_copy(rz_rep[32:42, :], rep[32:42, :])

            # mm2: both halves concurrently in PE column groups 0 and 1
            op0 = o_ps.tile([O, HW], F32, tag="ops0")
            op1 = o_ps.tile([42, HW], F32, tag="ops1")
            outs = [op0, op1[32 : 32 + O, :]]
            for c in range(ND):
                for h in range(NH):
                    nc.tensor.matmul(
                        outs[h],
                        lhsT=ec[:, c, :],
                        rhs=lt[:, c, h * HW : (h + 1) * HW],
                        start=(c == 0),
                        stop=(c == ND - 1),
                        tile_position=(0, 32 * h),
                    )
            # fin: scale by 1/Z during PSUM->SBUF copy; halves on different
            # engines so they run concurrently.
            o_sb = osb_pool.tile([42, T], F32, tag="osb")
            out2d = out[s].rearrange("(o t) -> o t", o=O)
            nc.scalar.activation(
                out=o_sb[0:O, 0:HW], in_=outs[0],
                func=mybir.ActivationFunctionType.Copy, scale=rz,
            )
            nc.sync.dma_start(out=out2d[:, 0:HW], in_=o_sb[0:O, 0:HW])
            if NH == 2:
                nc.vector.tensor_scalar_mul(
                    o_sb[32:42, HW:T], outs[1], rz_rep[32:42, :]
                )
                nc.sync.dma_start(out=out2d[:, HW:T], in_=o_sb[32:42, HW:T])

        for s in range(b_per):
            process_sample(s)

    nc.compile()
    return nc


_NC = None
TRACE = False
LAST_RESULT = None
BUILD_KWARGS = {}


def _get_nc():
    global _NC
    if _NC is None:
        _NC = build_nc(**BUILD_KWARGS)
    return _NC


def kernel(logits, decision, W, b):
    """Full-input entry point: shards batch over 8 cores, returns (16, 10240)."""
    global LAST_RESULT
    logits = np.asarray(logits, dtype=np.float32)
    W = np.asarray(W, dtype=np.float32)
    b = np.asarray(b, dtype=np.float32)
    nc = _get_nc()
    bp = B_FULL // N_CORES
    in_maps = [
        {"logits": np.ascontiguousarray(logits[i * bp : (i + 1) * bp]), "W": W, "b": b}
        for i in range(N_CORES)
    ]
    res = run_bass_kernel_spmd(nc, in_maps, core_ids=list(range(N_CORES)), trace=TRACE)
    LAST_RESULT = res
    return np.concatenate([res.results[i]["out"] for i in range(N_CORES)], axis=0)


# revision 19
# speedup vs baseline: 1.0800x; 1.0800x over previous
"""Trainium2 Bass/Tile kernel for ExtAttentionPool (nn_ExtAttentionPool).

Math (per sample b):
    S[u, o]  = sum_d L[u, d] * W[o, d]
    E[o, u]  = exp(S[u,o]/O + b[o]/O)          (softmax numerator over u)
    Z[o]     = sum_u E[o, u]
    OUT[o,t] = (1/Z[o]) * sum_c E[o, c] * L[t, c]
    result row b = OUT flattened (O-major), shape (O*T,)

Sharding: data-parallel over batch B=16 across 8 cores (2 samples/core).

Key implementation points:
  - Both matmuls contract over logits' D axis, so logits is transposed
    on-chip as a REGULAR bf16 matmul against an identity moving operand
    (engages fast-weight-load, counts as PE-busy for the HAM clock gate).
  - Logits are cast f32->bf16 inline in the SWDGE DMA load; accumulation
    stays fp32 in PSUM. All chunk DMAs are issued up front; chunks are
    kept <= 1 MiB so arrival gaps stay under the ~3.4 us HAM re-throttle
    window and the PE clock stays at 2.4 GHz.
  - mm1 is emitted in 256-wide column quarters as soon as the t-rows
    feeding a quarter are transposed: it fills PE idle gaps during the
    DMA stream and leaves only one small quarter on the critical tail.
  - mm2's two 512-wide halves run concurrently in PE column groups 0/1
    (tile_position). The 1/Z scaling rides the final PSUM->SBUF copies:
    half 0 scales by rz on ScalarE, half 1 by a partition-shifted rz
    (tiny const shift-matrix matmul) on VectorE.
"""

import numpy as np
from contextlib import ExitStack

import concourse.bass as bass
import concourse.mybir as mybir
import concourse.tile as tile
from concourse import bacc
from concourse.bass_utils import run_bass_kernel_spmd
from concourse.masks import make_identity

F32 = mybir.dt.float32
BF16 = mybir.dt.bfloat16

N_CORES = 8
B_FULL = 16


def build_nc(b_per=2, T=1024, D=1024, O=10, warmup_mms=36):
    """Build the per-core Bass program (bf16 compute). Same on all 8 cores."""
    P = 128
    NT = T // P            # 128-row t-blocks
    ND = D // P            # 128-col d-blocks
    QW = min(T, 256)       # mm1 quarter width
    NQ = T // QW           # mm1 quarters
    NH = max(1, T // 512)  # mm2 512-wide halves
    HW = min(T, 512)
    # per-sample DMA chunk plans (in 128-row blocks); all chunks <= 1 MiB.
    if NT == 8:
        plans = [[1, 1, 2, 2, 2], [2, 2, 2, 1, 1]]
    else:
        plans = [[1] * NT for _ in range(b_per)]

    nc = bacc.Bacc(
        "TRN2", target_bir_lowering=False, debug=False, enable_asserts=False
    )
    logits = nc.dram_tensor("logits", (b_per, T, D), F32, kind="ExternalInput").ap()
    w_in = nc.dram_tensor("W", (O, D), F32, kind="ExternalInput").ap()
    b_in = nc.dram_tensor("b", (O,), F32, kind="ExternalInput").ap()
    out = nc.dram_tensor("out", (b_per, O * T), F32, kind="ExternalOutput").ap()

    n_chunks = sum(len(p) for p in plans)

    with tile.TileContext(nc) as tc, ExitStack() as ctx:
        singles = ctx.enter_context(tc.tile_pool(name="singles", bufs=1))
        lr_pool = ctx.enter_context(tc.tile_pool(name="lr", bufs=n_chunks))
        lt_pool = ctx.enter_context(tc.tile_pool(name="lt", bufs=2))
        e_pool = ctx.enter_context(tc.tile_pool(name="e", bufs=2))
        z_pool = ctx.enter_context(tc.tile_pool(name="z", bufs=2))
        osb_pool = ctx.enter_context(tc.tile_pool(name="osb", bufs=2))
        slab_ps = ctx.enter_context(tc.tile_pool(name="slab", bufs=3, space="PSUM"))
        s_ps = ctx.enter_context(tc.tile_pool(name="sps", bufs=2, space="PSUM"))
        o_ps = ctx.enter_context(tc.tile_pool(name="ops", bufs=1, space="PSUM"))
        et_ps = ctx.enter_context(tc.tile_pool(name="etps", bufs=1, space="PSUM"))

        # --- identity (f32 master + bf16 cast), then chunk DMAs up front ---
        identf = singles.tile([P, P], F32)
        make_identity(nc, identf)
        ident = singles.tile([P, P], BF16)
        nc.vector.tensor_copy(ident, identf)

        max_rj = max(max(p) for p in plans)
        lr_tiles = {}  # (s, chunk_idx) -> (lr_tile, r0, rj, is_f32)
        w_sb = singles.tile([O, D], BF16)
        for s in range(b_per):
            r = 0
            for ci, rj in enumerate(plans[s]):
                f32_chunk = False
                if f32_chunk:
                    # HWDGE (sync) f32 load: lower first-byte latency, and
                    # the f32 transposes double as HAM warmup.
                    lr = lr_pool.tile(
                        [P, rj, D], F32, tag="lrf", name=f"lrf_s{s}c{ci}"
                    )
                    nc.sync.dma_start(
                        out=lr,
                        in_=logits[
                            s, r * P : (r + rj) * P, :
                        ].rearrange("(j p) d -> p j d", p=P),
                    )
                else:
                    lr = lr_pool.tile(
                        [P, max_rj, D], BF16, tag="lr", name=f"lr_s{s}c{ci}"
                    )
                    nc.gpsimd.dma_start(
                        out=lr[:, :rj, :],
                        in_=logits[
                            s, r * P : (r + rj) * P, :
                        ].rearrange("(j p) d -> p j d", p=P),
                    )
                lr_tiles[(s, ci)] = (lr, r, rj, f32_chunk)
                r += rj
                if s == 0 and ci == 0:
                    # W load early (needed by the WT transposes)
                    nc.gpsimd.dma_start(out=w_sb, in_=w_in)

        b_sb = singles.tile([O, 1], F32)
        nc.sync.dma_start(out=b_sb, in_=b_in.rearrange("(o u) -> o u", u=1))
        bias01 = singles.tile([O, 1], F32)
        nc.scalar.activation(
            out=bias01, in_=b_sb,
            func=mybir.ActivationFunctionType.Copy, scale=1.0 / O,
        )

        # shiftmat[o, m] = 1 iff m == o or m == o + 32 (for rz replication)
        shiftmat = singles.tile([O, 42], F32)
        nc.gpsimd.memset(shiftmat, 0.0)
        nc.gpsimd.affine_select(
            out=shiftmat, in_=shiftmat,
            compare_op=mybir.AluOpType.not_equal, fill=1.0,
            base=0, pattern=[[-1, 42]], channel_multiplier=1,
        )
        nc.gpsimd.affine_select(
            out=shiftmat, in_=shiftmat,
            compare_op=mybir.AluOpType.not_equal, fill=1.0,
            base=32, pattern=[[-1, 42]], channel_multiplier=1,
        )

        # --- PE warmup: identity matmuls to lift the HAM clock gate ---
        warm = slab_ps.tile([P, 4 * P], F32, tag="slab")
        for i in range(warmup_mms):
            k = i % 4
            nc.tensor.matmul(
                warm[:, k * P : (k + 1) * P], lhsT=ident, rhs=ident,
                start=True, stop=True,
            )

        # WT[dp, c, o] = W[o, 128c+dp]  (regular-matmul transpose)
        wt_stage = et_ps.tile([P, ND, O], F32, tag="etps")
        for c in range(ND):
            nc.tensor.matmul(
                wt_stage[:, c, :],
                lhsT=w_sb[:, c * P : (c + 1) * P],
                rhs=ident[:O, :O],
                start=True, stop=True,
            )
        wt_sb = singles.tile([P, ND, O], BF16)
        nc.vector.tensor_copy(wt_sb, wt_stage)

        i_copy = [0]

        def transpose_blocks(lr, lt, j, r, is_f32=False):
            """PE-transpose row-block r (from lr slot j) into lt."""
            rhs_id = identf if is_f32 else ident
            for g in range(ND // 4):
                slab = slab_ps.tile([P, 4 * P], F32, tag="slab")
                for k in range(4):
                    c = 4 * g + k
                    nc.tensor.matmul(
                        slab[:, k * P : (k + 1) * P],
                        lhsT=lr[:, j, c * P : (c + 1) * P],
                        rhs=rhs_id,
                        start=True, stop=True,
                    )
                dst = lt[:, 4 * g : 4 * g + 4, r * P : (r + 1) * P]
                if i_copy[0] % 5 < 3:
                    nc.vector.tensor_copy(dst, slab)
                else:
                    nc.scalar.activation(
                        out=dst, in_=slab,
                        func=mybir.ActivationFunctionType.Copy,
                    )
                i_copy[0] += 1

        def process_sample(s):
            lt = lt_pool.tile([P, ND, T], BF16, tag="lt")
            e_sb = e_pool.tile([O, T], BF16, tag="e")
            zparts = z_pool.tile([O, NQ], F32, tag="z")
            et_stage = et_ps.tile([P, ND, O], F32, tag="etps")
            ec = e_pool.tile([P, ND, O], BF16, tag="ec")

            r = 0
            q_done = 0
            for ci, rj in enumerate(plans[s]):
                lr, r0, _, is_f32 = lr_tiles[(s, ci)]
                for j in range(rj):
                    transpose_blocks(lr, lt, j, r0 + j, is_f32)
                r += rj
                # mm1 + exp + E-transpose for every 256-wide quarter whose
                # t-rows are now fully transposed
                while q_done < NQ and r * P >= (q_done + 1) * QW:
                    q = q_done
                    sp = s_ps.tile([O, QW], F32, tag="sps", name=f"sp{s}_{q}")
                    for c in range(ND):
                        nc.tensor.matmul(
                            sp,
                            lhsT=wt_sb[:, c, :],
                            rhs=lt[:, c, q * QW : (q + 1) * QW],
                            start=(c == 0),
                            stop=(c == ND - 1),
                        )
                    nc.scalar.activation(
                        out=e_sb[:, q * QW : (q + 1) * QW],
                        in_=sp,
                        func=mybir.ActivationFunctionType.Exp,
                        scale=1.0 / O,
                        bias=bias01,
                        accum_out=zparts[:, q : q + 1],
                    )
                    cpq = QW // P  # E-transpose blocks in this quarter
                    for c in range(q * cpq, (q + 1) * cpq):
                        nc.tensor.matmul(
                            et_stage[:, c, :],
                            lhsT=e_sb[:, c * P : (c + 1) * P],
                            rhs=ident[:O, :O],
                            start=True, stop=True,
                        )
                    nc.vector.tensor_copy(
                        ec[:, q * cpq : (q + 1) * cpq, :],
                        et_stage[:, q * cpq : (q + 1) * cpq, :],
                    )
                    q_done += 1

            # softmax denominator chain
            zsum = z_pool.tile([O, 1], F32, tag="zs")
            nc.vector.reduce_sum(zsum, zparts, axis=mybir.AxisListType.X)
            rz = z_pool.tile([O, 1], F32, tag="rz")
            nc.vector.reciprocal(rz, zsum)
            if NH == 2:
                # replicate rz to partitions 32:32+O for the packed half-1
                rep = et_ps.tile([42, 1], F32, tag="etps")
                nc.tensor.matmul(rep, lhsT=shiftmat, rhs=rz, start=True, stop=True)
                rz_rep = z_pool.tile([42, 1], F32, tag="rzrep")
                nc.vector.tensor_copy(rz_rep[32:42, :], rep[32:42, :])

            # mm2: both halves concurrently in PE column groups 0 and 1
            op0 = o_ps.tile([O, HW], F32, tag="ops0")
            op1 = o_ps.tile([42, HW], F32, tag="ops1")
            outs = [op0, op1[32 : 32 + O, :]]
            for c in range(ND):
                for h in range(NH):
                    nc.tensor.matmul(
                        outs[h],
                        lhsT=ec[:, c, :],
                        rhs=lt[:, c, h * HW : (h + 1) * HW],
                        start=(c == 0),
                        stop=(c == ND - 1),
                        tile_position=(0, 32 * h),
                    )
            # fin: scale by 1/Z during PSUM->SBUF copy; halves on different
            # engines so they run concurrently.
            o_sb = osb_pool.tile([42, T], F32, tag="osb")
            out2d = out[s].rearrange("(o t) -> o t", o=O)
            nc.scalar.activation(
                out=o_sb[0:O, 0:HW], in_=outs[0],
                func=mybir.ActivationFunctionType.Copy, scale=rz,
            )
            nc.sync.dma_start(out=out2d[:, 0:HW], in_=o_sb[0:O, 0:HW])
            if NH == 2:
                nc.vector.tensor_scalar_mul(
                    o_sb[32:42, HW:T], outs[1], rz_rep[32:42, :]
                )
                nc.sync.dma_start(out=out2d[:, HW:T], in_=o_sb[32:42, HW:T])

        for s in range(b_per):
            process_sample(s)

    nc.compile()
    return nc


_NC = None
TRACE = False
LAST_RESULT = None
BUILD_KWARGS = {}


def _get_nc():
    global _NC
    if _NC is None:
        _NC = build_nc(**BUILD_KWARGS)
    return _NC


def kernel(logits, decision, W, b):
    """Full-input entry point: shards batch over 8 cores, returns (16, 10240)."""
    global LAST_RESULT
    logits = np.asarray(logits, dtype=np.float32)
    W = np.asarray(W, dtype=np.float32)
    b = np.asarray(b, dtype=np.float32)
    nc = _get_nc()
    bp = B_FULL // N_CORES
    in_maps = [
        {"logits": np.ascontiguousarray(logits[i * bp : (i + 1) * bp]), "W": W, "b": b}
        for i in range(N_CORES)
    ]
    res = run_bass_kernel_spmd(nc, in_maps, core_ids=list(range(N_CORES)), trace=TRACE)
    LAST_RESULT = res
    return np.concatenate([res.results[i]["out"] for i in range(N_CORES)], axis=0)
